# revision 6
# baseline (speedup 1.0000x reference)
"""CMSA (cross-modal self-attention) model on 8 Trainium2 NeuronCores.

Model (B=4, C=256, H=W=64, N=4096, A=256):
  spatial = fixed 8-channel coordinate features            [B, 8, H, W]
  mm   = concat(images, flows, spatial)                    [B, 520, H, W]
  img_feat  = CMSA(mm,   img_w*)                           [B, 256, H, W]
  lang_feat = CMSA(flows, lang_w*)                         [B, 256, H, W]
  out = conv1x1(concat(img_feat, lang_feat, spatial), fus) [B, 256, H, W]
where CMSA(x) = wo @ softmax((wt@x)^T (wp@x)) applied to (wv@x), all 1x1 convs.

Sharding: 8 cores = 4 samples x 2 halves of the N=4096 pixel axis.  Each core
computes both CMSA branches and the fused output for its 2048 columns,
flash-attention style (full 4096x4096 attention rows never materialized in
HBM).  Attention is computed in the "transposed" orientation LT[m, n] so that
softmax needs no PE transposes: exp is taken without max-subtraction (logits
are bounded ~|15| for this model scale, safe in f32), the denominator is a
ones-matmul over partitions, and the value bias bv is folded into an effective
output bias bo_eff = wo@bv + bo using softmax row-sum normalization.

All matmuls run as float32r (full PE rate, ~1e-4 relative error).
"""

import numpy as np

import concourse.bass as bass
import concourse.tile as tile
import concourse.mybir as mybir
from concourse import bacc
from concourse.bass_utils import run_bass_kernel_spmd

F32 = mybir.dt.float32
F32R = mybir.dt.float32r
AF = mybir.ActivationFunctionType
ALU = mybir.AluOpType

B = 4
H = W = 64
N = H * W            # 4096
NC = N // 2          # columns per core
A = 256
C_MM = 520
NB = 512             # psum column block
NSB = NC // NB       # 4 blocks per core chunk
MT = N // 128        # 32 m-tiles
KI = 5               # k-tiles for C=520 (4x128 + 8)
KL = 2               # k-tiles for C=256

_CACHE = {}


def _emit(nc, tc, T):
    """Emit the per-core program. T maps dram tensor names -> APs."""
    ones_f32 = None

    # ---- pools ---------------------------------------------------------
    # left stack: whole-kernel consts | theta/phi/VT (img then lang, tag-shared)
    pL1 = tc.alloc_tile_pool(name="consts", bufs=1, side="left")
    pL2 = tc.alloc_tile_pool(name="abc", bufs=1, side="left")
    # right stack: R1 mm23+spc (to end of lang) | R2 mm01+sp+img qkv w (img
    # qkv only) | R3 working set (attention + tails)
    pR1 = tc.alloc_tile_pool(name="mm23", bufs=1, side="right")
    pR2 = tc.alloc_tile_pool(name="mm01", bufs=1, side="right")
    pps = tc.alloc_tile_pool(name="ps", bufs=1, space="PSUM")

    # ---- consts --------------------------------------------------------
    ones32 = pL1.tile([128, 1], F32, tag="ones32")
    nc.vector.memset(ones32, 1.0)
    ones_r = pL1.tile([128, 1], F32R, tag="ones")
    nc.scalar.copy(out=ones_r, in_=ones32)
    bias_t = {}
    for nm in ("img_bt2", "img_bp2", "lang_bt2", "lang_bp2",
               "img_bo2", "lang_bo2", "fus_b2"):
        t = pL1.tile([128, 2], F32, tag=nm, name=nm)
        nc.sync.dma_start(out=t, in_=T[nm])
        bias_t[nm] = t
    part_out = pL1.tile([128, 2, NC], F32, tag="part_out")

    # ---- big inputs ----------------------------------------------------
    # Load order matters for PE warmup: img qkv weights and the spatial rows
    # first (every qkv psum chain ends on them), then mm in column chunks
    # breadth-first so the first qkv tiles can start after ~1/4 of the load.
    imgw = {}
    mm_sb = [None] * 4
    for k in (2, 3):
        mm_sb[k] = pR1.tile([128, N], F32R, tag=f"mm{k}", name=f"mm{k}")
    spc = pR1.tile([8, NC], F32R, tag="spc")
    for k in (0, 1):
        mm_sb[k] = pR2.tile([128, N], F32R, tag=f"mm{k}", name=f"mm{k}")
    sp_sb = pR2.tile([8, N], F32R, tag="sp")
    for nm in ("img_wtT", "img_wpT", "img_wvT"):
        imgw[nm] = pR2.tile([128, KI, A], F32R, tag=nm, name=nm)

    CS = N // 4
    def mm_cs_dma(cs):
        for k in range(4):
            nc.sync.dma_start(
                out=mm_sb[k][:, cs * CS:(cs + 1) * CS],
                in_=T["mm"][k * 128:(k + 1) * 128, cs * CS:(cs + 1) * CS].bitcast(F32R))

    nc.sync.dma_start(out=imgw["img_wtT"], in_=T["img_wtT"].bitcast(F32R))
    mm_cs_dma(0)
    nc.sync.dma_start(out=sp_sb, in_=T["mm"][512:520, :].bitcast(F32R))
    nc.sync.dma_start(out=imgw["img_wpT"], in_=T["img_wpT"].bitcast(F32R))
    nc.sync.dma_start(out=imgw["img_wvT"], in_=T["img_wvT"].bitcast(F32R))
    for cs in range(1, 4):
        mm_cs_dma(cs)

    def mm_ktile(k, cols):
        """[k-partitions, cols] slice of the mm operand for k-tile k."""
        if k < 4:
            return mm_sb[k][:, cols]
        return sp_sb[:, cols]

    def qkv_phase(branch, theta, phi, vt, wt, wp, wv, ks, bt2, bp2):
        """Computes theta [128,2,NC], phi [128,2,N], vt [128,MT,A] for one
        branch. ks = list of (ktile_idx, partitions)."""
        nk = len(ks)
        for a2 in range(2):
            asl = slice(a2 * 128, (a2 + 1) * 128)
            for ns in range(NSB):
                csl = slice(ns * NB, (ns + 1) * NB)
                q_ps = pps.tile([128, NB], F32, tag="blk", bufs=4, name="q_ps")
                for i, (k, kp) in enumerate(ks):
                    nc.tensor.matmul(q_ps, lhsT=wt[:kp, i, asl],
                                     rhs=mm_ktile(k, csl),
                                     start=(i == 0), stop=(i == nk - 1))
                nc.vector.tensor_scalar(out=theta[:, a2, csl], in0=q_ps,
                                        scalar1=bt2[:, a2:a2 + 1], scalar2=None,
                                        op0=ALU.add)
            for ns in range(N // NB):
                csl = slice(ns * NB, (ns + 1) * NB)
                q_ps = pps.tile([128, NB], F32, tag="blk", bufs=4, name="q_ps")
                for i, (k, kp) in enumerate(ks):
                    nc.tensor.matmul(q_ps, lhsT=wp[:kp, i, asl],
                                     rhs=mm_ktile(k, csl),
                                     start=(i == 0), stop=(i == nk - 1))
                nc.vector.tensor_scalar(out=phi[:, a2, csl], in0=q_ps,
                                        scalar1=bp2[:, a2:a2 + 1], scalar2=None,
                                        op0=ALU.add)
        for m in range(MT):
            msl = slice(m * 128, (m + 1) * 128)
            v_ps = pps.tile([128, A], F32, tag="blk", bufs=4, name="v_ps")
            for i, (k, kp) in enumerate(ks):
                nc.tensor.matmul(v_ps, lhsT=mm_ktile(k, msl)[:kp, :],
                                 rhs=wv[:kp, i, :],
                                 start=(i == 0), stop=(i == nk - 1))
            nc.vector.tensor_copy(out=vt[:, m, :], in_=v_ps)

    def attn_phase(branch, theta, phi, vt, pools):
        """Flash attention + wo conv + partial fusion for one branch."""
        woT = pools["woT_" + branch]
        fusT = pools["fusT"]
        bo2 = bias_t[f"{branch}_bo2"]
        pR3 = pools["pR3"]
        for nb in range(NSB):
            csl = slice(nb * NB, (nb + 1) * NB)
            att_ps = pps.tile([128, 2, NB], F32, tag="big2", bufs=2, name="att_ps")
            acc = pR3.tile([128, NB], F32R, tag="acc", bufs=2, name="acc")
            for m in range(MT):
                msl = slice(m * 128, (m + 1) * 128)
                lt_ps = pps.tile([128, NB], F32, tag="blk", bufs=4, name="lt_ps")
                for ka in range(2):
                    nc.tensor.matmul(lt_ps, lhsT=phi[:, ka, msl],
                                     rhs=theta[:, ka, csl],
                                     start=(ka == 0), stop=(ka == 1))
                p_sb = pR3.tile([128, NB], F32R, tag="p", bufs=3, name="p_sb")
                nc.scalar.activation(out=p_sb, in_=lt_ps, func=AF.Exp)
                if m == 0:
                    nc.vector.tensor_copy(out=acc, in_=p_sb)
                else:
                    nc.vector.tensor_add(out=acc, in0=acc, in1=p_sb)
                for a2 in range(2):
                    nc.tensor.matmul(att_ps[:, a2, :],
                                     lhsT=vt[:, m, a2 * 128:(a2 + 1) * 128],
                                     rhs=p_sb,
                                     start=(m == 0), stop=(m == MT - 1))
            # softmax denominator -> reciprocal -> broadcast
            rs_ps = pps.tile([128, NB], F32, tag="blk", bufs=4, name="rs_ps")
            nc.tensor.matmul(rs_ps[0:1, :], lhsT=ones_r, rhs=acc,
                             start=True, stop=True)
            rcp = pR3.tile([1, NB], F32, tag="rcp", bufs=2, name="rcp")
            nc.vector.reciprocal(out=rcp, in_=rs_ps[0:1, :])
            bc = pR3.tile([128, NB], F32, tag="bc", bufs=2, name="bc")
            nc.gpsimd.partition_broadcast(bc, rcp)
            # att (unnormalized) back to sbuf for the wo conv
            att_sb = pR3.tile([128, 2, NB], F32R, tag="att_sb", bufs=2, name="att_sb")
            for a2 in range(2):
                nc.scalar.copy(out=att_sb[:, a2, :], in_=att_ps[:, a2, :])
            wo_ps = pps.tile([128, 2, NB], F32, tag="big2", bufs=2, name="wo_ps")
            for o2 in range(2):
                for ka in range(2):
                    nc.tensor.matmul(wo_ps[:, o2, :],
                                     lhsT=woT[:, ka, o2 * 128:(o2 + 1) * 128],
                                     rhs=att_sb[:, ka, :],
                                     start=(ka == 0), stop=(ka == 1))
            feat = pR3.tile([128, 2, NB], F32R, tag="feat", bufs=2, name="feat")
            for o2 in range(2):
                t1 = pR3.tile([128, NB], F32, tag="t1", bufs=2, name="t1")
                nc.vector.tensor_tensor(out=t1, in0=wo_ps[:, o2, :], in1=bc, op=ALU.mult)
                nc.scalar.activation(out=feat[:, o2, :], in_=t1,
                                     func=AF.Identity, bias=bo2[:, o2:o2 + 1])
            # partial fusion
            f_ps = pps.tile([128, 2, NB], F32, tag="big2", bufs=2, name="f_ps")
            if branch == "img":
                for q2 in range(2):
                    qsl = slice(q2 * 128, (q2 + 1) * 128)
                    for k2 in range(2):
                        nc.tensor.matmul(f_ps[:, q2, :], lhsT=fusT[:, k2, qsl],
                                         rhs=feat[:, k2, :],
                                         start=(k2 == 0), stop=False)
                    nc.tensor.matmul(f_ps[:, q2, :], lhsT=fusT[:8, 4, qsl],
                                     rhs=spc[:, csl], start=False, stop=True)
                    nc.scalar.activation(out=part_out[:, q2, csl], in_=f_ps[:, q2, :],
                                         func=AF.Identity,
                                         bias=bias_t["fus_b2"][:, q2:q2 + 1])
            else:
                for q2 in range(2):
                    qsl = slice(q2 * 128, (q2 + 1) * 128)
                    for k2 in range(2):
                        nc.tensor.matmul(f_ps[:, q2, :], lhsT=fusT[:, 2 + k2, qsl],
                                         rhs=feat[:, k2, :],
                                         start=(k2 == 0), stop=(k2 == 1))
                    out_t = pR3.tile([128, NB], F32, tag="out_t", bufs=2, name="out_t")
                    nc.vector.tensor_tensor(out=out_t, in0=f_ps[:, q2, :],
                                            in1=part_out[:, q2, csl], op=ALU.add)
                    nc.sync.dma_start(
                        out=T["out"][q2 * 128:(q2 + 1) * 128, csl], in_=out_t)

    # ---- img qkv -------------------------------------------------------
    theta = pL2.tile([128, 2, NC], F32R, tag="theta", name="theta_i")
    phi = pL2.tile([128, 2, N], F32R, tag="phi", name="phi_i")
    vt = pL2.tile([128, MT, A], F32R, tag="vt", name="vt_i")
    ks_img = [(0, 128), (1, 128), (2, 128), (3, 128), (4, 8)]
    qkv_phase("img", theta, phi, vt, imgw["img_wtT"], imgw["img_wpT"],
              imgw["img_wvT"], ks_img, bias_t["img_bt2"], bias_t["img_bp2"])
    # stash this core's spatial columns before sp_sb dies with pR2
    nc.scalar.copy(out=spc, in_=sp_sb[:, 0:NC])
    pR2.release()

    # ---- working pool (attention + tails) ------------------------------
    pR3 = tc.alloc_tile_pool(name="work", bufs=1, side="right")
    pools = {"pR3": pR3}
    for nm, kt in (("woT_img", 2), ("woT_lang", 2), ("fusT", KI),
                   ("lang_wtT", KL), ("lang_wpT", KL), ("lang_wvT", KL)):
        dnm = {"woT_img": "img_woT", "woT_lang": "lang_woT"}.get(nm, nm)
        t = pR3.tile([128, kt, A], F32R, tag=nm, name=nm)
        nc.sync.dma_start(out=t, in_=T[dnm].bitcast(F32R))
        pools[nm] = t

    # ---- img attention + partial fusion --------------------------------
    attn_phase("img", theta, phi, vt, pools)

    # ---- lang qkv ------------------------------------------------------
    theta_l = pL2.tile([128, 2, NC], F32R, tag="theta", name="theta_l")
    phi_l = pL2.tile([128, 2, N], F32R, tag="phi", name="phi_l")
    vt_l = pL2.tile([128, MT, A], F32R, tag="vt", name="vt_l")
    ks_lang = [(2, 128), (3, 128)]
    qkv_phase("lang", theta_l, phi_l, vt_l, pools["lang_wtT"],
              pools["lang_wpT"], pools["lang_wvT"], ks_lang,
              bias_t["lang_bt2"], bias_t["lang_bp2"])

    # ---- lang attention + final output ---------------------------------
    attn_phase("lang", theta_l, phi_l, vt_l, pools)

    pR3.release()
    pR1.release()
    pL2.release()
    pL1.release()
    pps.release()


def _build(repeat=1):
    nc = bacc.Bacc("TRN2", target_bir_lowering=False, debug=False, num_devices=8)
    T = {}
    T["mm"] = nc.dram_tensor("mm", [C_MM, N], F32, kind="ExternalInput").ap()
    for nm in ("img_wtT", "img_wpT", "img_wvT", "fusT"):
        T[nm] = nc.dram_tensor(nm, [128, KI, A], F32, kind="ExternalInput").ap()
    for nm in ("lang_wtT", "lang_wpT", "lang_wvT"):
        T[nm] = nc.dram_tensor(nm, [128, KL, A], F32, kind="ExternalInput").ap()
    for nm in ("img_woT", "lang_woT"):
        T[nm] = nc.dram_tensor(nm, [128, 2, A], F32, kind="ExternalInput").ap()
    for nm in ("img_bt2", "img_bp2", "lang_bt2", "lang_bp2",
               "img_bo2", "lang_bo2", "fus_b2"):
        T[nm] = nc.dram_tensor(nm, [128, 2], F32, kind="ExternalInput").ap()
    T["out"] = nc.dram_tensor("out", [A, NC], F32, kind="ExternalOutput").ap()

    with tile.TileContext(nc) as tc:
        for _ in range(repeat):
            _emit(nc, tc, T)
    nc.compile()
    return nc


def _spatial():
    gy, gx = np.meshgrid(np.linspace(0, 1, H, dtype=np.float32),
                         np.linspace(0, 1, W, dtype=np.float32), indexing="ij")
    feats = [gx, gy, 1.0 - gx, 1.0 - gy] + [(gx + gy) * 0.5] * 4
    return np.stack(feats[:8], axis=0).reshape(8, N).astype(np.float32)


def _pack_kT(wT, kt):
    """[C, A] (pre-transposed weight) -> [128, kt, A] partition-tiled."""
    out = np.zeros((128, kt, wT.shape[1]), np.float32)
    for k in range(kt):
        rows = wT[k * 128:min((k + 1) * 128, wT.shape[0])]
        out[:rows.shape[0], k] = rows
    return out


def _bias2(b):
    return np.ascontiguousarray(b.reshape(2, 128).T)


def _in_maps(inputs):
    f = lambda k: np.asarray(inputs[k], np.float32)
    images, flows = f("images"), f("flows")
    sp = _spatial()

    base = {
        "img_wtT": _pack_kT(f("img_wt").T, KI),
        "img_wpT": _pack_kT(f("img_wp").T, KI),
        "img_wvT": _pack_kT(f("img_wv").T, KI),
        "fusT": _pack_kT(f("fus_w").T, KI),
        "lang_wtT": _pack_kT(f("lang_wt").T, KL),
        "lang_wpT": _pack_kT(f("lang_wp").T, KL),
        "lang_wvT": _pack_kT(f("lang_wv").T, KL),
        "img_woT": _pack_kT(f("img_wo").T, 2),
        "lang_woT": _pack_kT(f("lang_wo").T, 2),
        "img_bt2": _bias2(f("img_bt")),
        "img_bp2": _bias2(f("img_bp")),
        "lang_bt2": _bias2(f("lang_bt")),
        "lang_bp2": _bias2(f("lang_bp")),
        "img_bo2": _bias2(f("img_wo") @ f("img_bv") + f("img_bo")),
        "lang_bo2": _bias2(f("lang_wo") @ f("lang_bv") + f("lang_bo")),
        "fus_b2": _bias2(f("fus_b")),
    }

    in_maps = []
    for c in range(8):
        b, half = c // 2, c % 2
        mm = np.concatenate(
            [images[b].reshape(256, N), flows[b].reshape(256, N), sp], axis=0)
        if half:
            mm = np.roll(mm, -NC, axis=1)
        in_maps.append({**base, "mm": np.ascontiguousarray(mm)})
    return in_maps


def kernel(**inputs):
    if "nc" not in _CACHE:
        _CACHE["nc"] = _build()
    nc = _CACHE["nc"]
    in_maps = _in_maps(inputs)
    res = run_bass_kernel_spmd(nc, in_maps, list(range(8)))
    out = np.empty((B, A, N), np.float32)
    for c in range(8):
        b, half = c // 2, c % 2
        out[b][:, half * NC:(half + 1) * NC] = res.results[c]["out"]
    return out.reshape(B, A, H, W)


# revision 11
# speedup vs baseline: 1.1840x; 1.1840x over previous
"""CMSA (cross-modal self-attention) model on 8 Trainium2 NeuronCores.

Model (B=4, C=256, H=W=64, N=4096, A=256):
  spatial = fixed 8-channel coordinate features            [B, 8, H, W]
  mm   = concat(images, flows, spatial)                    [B, 520, H, W]
  img_feat  = CMSA(mm,   img_w*)                           [B, 256, H, W]
  lang_feat = CMSA(flows, lang_w*)                         [B, 256, H, W]
  out = conv1x1(concat(img_feat, lang_feat, spatial), fus) [B, 256, H, W]
where CMSA(x) = wo @ softmax((wt@x)^T (wp@x)) applied to (wv@x), all 1x1 convs.

Sharding: 8 cores = 4 samples x 2 halves of the N=4096 pixel axis.  Each core
computes both CMSA branches and the fused output for its 2048 columns,
flash-attention style (full 4096x4096 attention rows never materialized in
HBM).  Attention is computed in the "transposed" orientation LT[m, n] so that
softmax needs no PE transposes: exp is taken without max-subtraction (logits
are bounded ~|15| for this model scale, safe in f32), the denominator is a
ones-matmul over partitions, and the value bias bv is folded into an effective
output bias bo_eff = wo@bv + bo using softmax row-sum normalization.

All matmuls run as float32r (full PE rate, ~1e-4 relative error).
"""

import numpy as np

import concourse.bass as bass
import concourse.tile as tile
import concourse.mybir as mybir
from concourse import bacc
from concourse.bass_utils import run_bass_kernel_spmd

F32 = mybir.dt.float32
F32R = mybir.dt.float32r
AF = mybir.ActivationFunctionType
ALU = mybir.AluOpType

B = 4
H = W = 64
N = H * W            # 4096
NC = N // 2          # columns per core
A = 256
C_MM = 520
NB = 512             # psum column block
NSB = NC // NB       # 4 blocks per core chunk
MT = N // 128        # 32 m-tiles
KI = 5               # k-tiles for C=520 (4x128 + 8)
KL = 2               # k-tiles for C=256

_CACHE = {}


def _emit(nc, tc, T):
    """Emit the per-core program. T maps dram tensor names -> APs."""
    ones_f32 = None

    # ---- pools ---------------------------------------------------------
    # left stack: whole-kernel consts | theta/phi/VT (img then lang, tag-shared)
    pL1 = tc.alloc_tile_pool(name="consts", bufs=1, side="left")
    pL2 = tc.alloc_tile_pool(name="abc", bufs=1, side="left")
    # right stack: R1 mm23+spc (to end of lang) | R2 mm01+sp+img qkv w (img
    # qkv only) | R3 working set (attention + tails)
    pR1 = tc.alloc_tile_pool(name="mm23", bufs=1, side="right")
    pR2 = tc.alloc_tile_pool(name="mm01", bufs=1, side="right")
    pps = tc.alloc_tile_pool(name="ps", bufs=1, space="PSUM")

    # ---- consts --------------------------------------------------------
    ones32 = pL1.tile([128, 1], F32, tag="ones32")
    nc.vector.memset(ones32, 1.0)
    ones_r = pL1.tile([128, 1], F32R, tag="ones")
    nc.scalar.copy(out=ones_r, in_=ones32)
    bias_t = {}
    for nm in ("img_bt2", "img_bp2", "lang_bt2", "lang_bp2",
               "img_bo2", "lang_bo2", "fus_b2"):
        t = pL1.tile([128, 2], F32, tag=nm, name=nm)
        nc.sync.dma_start(out=t, in_=T[nm])
        bias_t[nm] = t
    part_out = pL1.tile([128, 2, NC], F32, tag="part_out")

    # ---- big inputs ----------------------------------------------------
    # Load order matters for PE warmup: img qkv weights and the spatial rows
    # first (every qkv psum chain ends on them), then mm in column chunks
    # breadth-first so the first qkv tiles can start after ~1/4 of the load.
    imgw = {}
    CS = N // 4
    # mm lives as [128, CS] chunk tiles so DMA->compute deps are exact
    mm_cs = [[None] * 4 for _ in range(4)]   # [k][cs]
    for k in (2, 3):
        for cs in range(4):
            mm_cs[k][cs] = pR1.tile([128, CS], F32R, tag=f"mm{k}c{cs}",
                                    name=f"mm{k}c{cs}")
    spc = pR1.tile([8, NC], F32R, tag="spc")
    for k in (0, 1):
        for cs in range(4):
            mm_cs[k][cs] = pR2.tile([128, CS], F32R, tag=f"mm{k}c{cs}",
                                    name=f"mm{k}c{cs}")
    sp_sb = pR2.tile([8, N], F32R, tag="sp")
    for nm in ("img_wtT", "img_wpT", "img_wvT"):
        imgw[nm] = pR2.tile([128, KI, A], F32R, tag=nm, name=nm)

    def mm_cs_dma(cs):
        for k in range(4):
            nc.sync.dma_start(
                out=mm_cs[k][cs],
                in_=T["mm"][k * 128:(k + 1) * 128, cs * CS:(cs + 1) * CS].bitcast(F32R))

    nc.sync.dma_start(out=imgw["img_wtT"], in_=T["img_wtT"].bitcast(F32R))
    mm_cs_dma(0)
    nc.sync.dma_start(out=sp_sb, in_=T["mm"][512:520, :].bitcast(F32R))
    mm_cs_dma(1)
    nc.sync.dma_start(out=imgw["img_wpT"], in_=T["img_wpT"].bitcast(F32R))
    mm_cs_dma(2)
    nc.sync.dma_start(out=imgw["img_wvT"], in_=T["img_wvT"].bitcast(F32R))
    mm_cs_dma(3)

    def mm_ktile(k, cols):
        """[k-partitions, cols] slice of the mm operand for k-tile k.
        cols must lie within one CS-sized chunk for k < 4."""
        if k == 4:
            return sp_sb[:, cols]
        cs, lo, hi = cols.start // CS, cols.start % CS, None
        assert cols.stop - cols.start <= CS and cols.stop <= (cs + 1) * CS
        return mm_cs[k][cs][:, lo:lo + (cols.stop - cols.start)]

    def qkv_phase(branch, theta, phi, vt, wt, wp, wv, ks, bt2, bp2):
        """Computes theta [128,2,NC], phi [128,2,N], vt [128,MT,A] for one
        branch. ks = list of (ktile_idx, partitions)."""
        nk = len(ks)
        for a2 in range(2):
            asl = slice(a2 * 128, (a2 + 1) * 128)
            for ns in range(NSB):
                csl = slice(ns * NB, (ns + 1) * NB)
                q_ps = pps.tile([128, NB], F32, tag="blk", bufs=4, name="q_ps")
                for i, (k, kp) in enumerate(ks):
                    nc.tensor.matmul(q_ps, lhsT=wt[:kp, i, asl],
                                     rhs=mm_ktile(k, csl),
                                     start=(i == 0), stop=(i == nk - 1))
                nc.vector.tensor_scalar(out=theta[:, a2, csl], in0=q_ps,
                                        scalar1=bt2[:, a2:a2 + 1], scalar2=None,
                                        op0=ALU.add)
            for ns in range(N // NB):
                csl = slice(ns * NB, (ns + 1) * NB)
                q_ps = pps.tile([128, NB], F32, tag="blk", bufs=4, name="q_ps")
                for i, (k, kp) in enumerate(ks):
                    nc.tensor.matmul(q_ps, lhsT=wp[:kp, i, asl],
                                     rhs=mm_ktile(k, csl),
                                     start=(i == 0), stop=(i == nk - 1))
                nc.vector.tensor_scalar(out=phi[:, a2, csl], in0=q_ps,
                                        scalar1=bp2[:, a2:a2 + 1], scalar2=None,
                                        op0=ALU.add)
        for m in range(MT):
            msl = slice(m * 128, (m + 1) * 128)
            v_ps = pps.tile([128, A], F32, tag="blk", bufs=4, name="v_ps")
            for i, (k, kp) in enumerate(ks):
                nc.tensor.matmul(v_ps, lhsT=mm_ktile(k, msl)[:kp, :],
                                 rhs=wv[:kp, i, :],
                                 start=(i == 0), stop=(i == nk - 1))
            nc.vector.tensor_copy(out=vt[:, m, :], in_=v_ps)

    def attn_phase(branch, theta, phi, vt, pools):
        """Flash attention + wo conv + partial fusion for one branch."""
        woT = pools["woT_" + branch]
        fusT = pools["fusT"]
        bo2 = bias_t[f"{branch}_bo2"]
        pR3 = pools["pR3"]
        for nb in range(NSB):
            csl = slice(nb * NB, (nb + 1) * NB)
            att_ps = pps.tile([128, 2, NB], F32, tag="big2", bufs=2, name="att_ps")
            acc = pR3.tile([128, NB], F32R, tag="acc", bufs=2, name="acc")
            for m in range(MT):
                msl = slice(m * 128, (m + 1) * 128)
                lt_ps = pps.tile([128, NB], F32, tag="blk", bufs=4, name="lt_ps")
                for ka in range(2):
                    nc.tensor.matmul(lt_ps, lhsT=phi[:, ka, msl],
                                     rhs=theta[:, ka, csl],
                                     start=(ka == 0), stop=(ka == 1))
                p_sb = pR3.tile([128, NB], F32R, tag="p", bufs=5, name="p_sb")
                nc.scalar.activation(out=p_sb, in_=lt_ps, func=AF.Exp)
                if m == 0:
                    nc.vector.tensor_copy(out=acc, in_=p_sb)
                else:
                    nc.vector.tensor_add(out=acc, in0=acc, in1=p_sb)
                for a2 in range(2):
                    nc.tensor.matmul(att_ps[:, a2, :],
                                     lhsT=vt[:, m, a2 * 128:(a2 + 1) * 128],
                                     rhs=p_sb,
                                     start=(m == 0), stop=(m == MT - 1))
            # softmax denominator -> reciprocal -> broadcast
            rs_ps = pps.tile([128, NB], F32, tag="blk", bufs=4, name="rs_ps")
            nc.tensor.matmul(rs_ps[0:1, :], lhsT=ones_r, rhs=acc,
                             start=True, stop=True)
            rcp = pR3.tile([1, NB], F32, tag="rcp", bufs=2, name="rcp")
            nc.vector.reciprocal(out=rcp, in_=rs_ps[0:1, :])
            bc = pR3.tile([128, NB], F32, tag="bc", bufs=2, name="bc")
            nc.gpsimd.partition_broadcast(bc, rcp)
            # att (unnormalized) back to sbuf for the wo conv
            att_sb = pR3.tile([128, 2, NB], F32R, tag="att_sb", bufs=3, name="att_sb")
            for a2 in range(2):
                nc.scalar.copy(out=att_sb[:, a2, :], in_=att_ps[:, a2, :])
            wo_ps = pps.tile([128, 2, NB], F32, tag="big2", bufs=2, name="wo_ps")
            for o2 in range(2):
                for ka in range(2):
                    nc.tensor.matmul(wo_ps[:, o2, :],
                                     lhsT=woT[:, ka, o2 * 128:(o2 + 1) * 128],
                                     rhs=att_sb[:, ka, :],
                                     start=(ka == 0), stop=(ka == 1))
            feat = pR3.tile([128, 2, NB], F32R, tag="feat", bufs=2, name="feat")
            for o2 in range(2):
                t1 = pR3.tile([128, NB], F32, tag="t1", bufs=3, name="t1")
                nc.vector.tensor_tensor(out=t1, in0=wo_ps[:, o2, :], in1=bc, op=ALU.mult)
                nc.scalar.activation(out=feat[:, o2, :], in_=t1,
                                     func=AF.Identity, bias=bo2[:, o2:o2 + 1])
            # partial fusion
            f_ps = pps.tile([128, 2, NB], F32, tag="big2", bufs=2, name="f_ps")
            if branch == "img":
                for q2 in range(2):
                    qsl = slice(q2 * 128, (q2 + 1) * 128)
                    for k2 in range(2):
                        nc.tensor.matmul(f_ps[:, q2, :], lhsT=fusT[:, k2, qsl],
                                         rhs=feat[:, k2, :],
                                         start=(k2 == 0), stop=False)
                    nc.tensor.matmul(f_ps[:, q2, :], lhsT=fusT[:8, 4, qsl],
                                     rhs=spc[:, csl], start=False, stop=True)
                    nc.scalar.activation(out=part_out[:, q2, csl], in_=f_ps[:, q2, :],
                                         func=AF.Identity,
                                         bias=bias_t["fus_b2"][:, q2:q2 + 1])
            else:
                for q2 in range(2):
                    qsl = slice(q2 * 128, (q2 + 1) * 128)
                    for k2 in range(2):
                        nc.tensor.matmul(f_ps[:, q2, :], lhsT=fusT[:, 2 + k2, qsl],
                                         rhs=feat[:, k2, :],
                                         start=(k2 == 0), stop=(k2 == 1))
                    out_t = pR3.tile([128, NB], F32, tag="out_t", bufs=2, name="out_t")
                    nc.vector.tensor_tensor(out=out_t, in0=f_ps[:, q2, :],
                                            in1=part_out[:, q2, csl], op=ALU.add)
                    nc.sync.dma_start(
                        out=T["out"][q2 * 128:(q2 + 1) * 128, csl], in_=out_t)

    # ---- img qkv -------------------------------------------------------
    theta = pL2.tile([128, 2, NC], F32R, tag="theta", name="theta_i")
    phi = pL2.tile([128, 2, N], F32R, tag="phi", name="phi_i")
    vt = pL2.tile([128, MT, A], F32R, tag="vt", name="vt_i")
    ks_img = [(0, 128), (1, 128), (2, 128), (3, 128), (4, 8)]
    qkv_phase("img", theta, phi, vt, imgw["img_wtT"], imgw["img_wpT"],
              imgw["img_wvT"], ks_img, bias_t["img_bt2"], bias_t["img_bp2"])
    # stash this core's spatial columns before sp_sb dies with pR2
    nc.scalar.copy(out=spc, in_=sp_sb[:, 0:NC])
    pR2.release()

    # ---- working pool (attention + tails) ------------------------------
    pR3 = tc.alloc_tile_pool(name="work", bufs=1, side="right")
    pools = {"pR3": pR3}
    for nm, kt in (("woT_img", 2), ("woT_lang", 2), ("fusT", KI),
                   ("lang_wtT", KL), ("lang_wpT", KL), ("lang_wvT", KL)):
        dnm = {"woT_img": "img_woT", "woT_lang": "lang_woT"}.get(nm, nm)
        t = pR3.tile([128, kt, A], F32R, tag=nm, name=nm)
        nc.sync.dma_start(out=t, in_=T[dnm].bitcast(F32R))
        pools[nm] = t

    # ---- img attention + partial fusion --------------------------------
    attn_phase("img", theta, phi, vt, pools)

    # ---- lang qkv ------------------------------------------------------
    theta_l = pL2.tile([128, 2, NC], F32R, tag="theta", name="theta_l")
    phi_l = pL2.tile([128, 2, N], F32R, tag="phi", name="phi_l")
    vt_l = pL2.tile([128, MT, A], F32R, tag="vt", name="vt_l")
    ks_lang = [(2, 128), (3, 128)]
    qkv_phase("lang", theta_l, phi_l, vt_l, pools["lang_wtT"],
              pools["lang_wpT"], pools["lang_wvT"], ks_lang,
              bias_t["lang_bt2"], bias_t["lang_bp2"])

    # ---- lang attention + final output ---------------------------------
    attn_phase("lang", theta_l, phi_l, vt_l, pools)

    pR3.release()
    pR1.release()
    pL2.release()
    pL1.release()
    pps.release()


def _build(repeat=1):
    nc = bacc.Bacc("TRN2", target_bir_lowering=False, debug=False, num_devices=8)
    T = {}
    T["mm"] = nc.dram_tensor("mm", [C_MM, N], F32, kind="ExternalInput").ap()
    for nm in ("img_wtT", "img_wpT", "img_wvT", "fusT"):
        T[nm] = nc.dram_tensor(nm, [128, KI, A], F32, kind="ExternalInput").ap()
    for nm in ("lang_wtT", "lang_wpT", "lang_wvT"):
        T[nm] = nc.dram_tensor(nm, [128, KL, A], F32, kind="ExternalInput").ap()
    for nm in ("img_woT", "lang_woT"):
        T[nm] = nc.dram_tensor(nm, [128, 2, A], F32, kind="ExternalInput").ap()
    for nm in ("img_bt2", "img_bp2", "lang_bt2", "lang_bp2",
               "img_bo2", "lang_bo2", "fus_b2"):
        T[nm] = nc.dram_tensor(nm, [128, 2], F32, kind="ExternalInput").ap()
    T["out"] = nc.dram_tensor("out", [A, NC], F32, kind="ExternalOutput").ap()

    with tile.TileContext(nc) as tc:
        for _ in range(repeat):
            _emit(nc, tc, T)
    nc.compile()
    return nc


def _spatial():
    gy, gx = np.meshgrid(np.linspace(0, 1, H, dtype=np.float32),
                         np.linspace(0, 1, W, dtype=np.float32), indexing="ij")
    feats = [gx, gy, 1.0 - gx, 1.0 - gy] + [(gx + gy) * 0.5] * 4
    return np.stack(feats[:8], axis=0).reshape(8, N).astype(np.float32)


def _pack_kT(wT, kt):
    """[C, A] (pre-transposed weight) -> [128, kt, A] partition-tiled."""
    out = np.zeros((128, kt, wT.shape[1]), np.float32)
    for k in range(kt):
        rows = wT[k * 128:min((k + 1) * 128, wT.shape[0])]
        out[:rows.shape[0], k] = rows
    return out


def _bias2(b):
    return np.ascontiguousarray(b.reshape(2, 128).T)


def _in_maps(inputs):
    f = lambda k: np.asarray(inputs[k], np.float32)
    images, flows = f("images"), f("flows")
    sp = _spatial()

    base = {
        "img_wtT": _pack_kT(f("img_wt").T, KI),
        "img_wpT": _pack_kT(f("img_wp").T, KI),
        "img_wvT": _pack_kT(f("img_wv").T, KI),
        "fusT": _pack_kT(f("fus_w").T, KI),
        "lang_wtT": _pack_kT(f("lang_wt").T, KL),
        "lang_wpT": _pack_kT(f("lang_wp").T, KL),
        "lang_wvT": _pack_kT(f("lang_wv").T, KL),
        "img_woT": _pack_kT(f("img_wo").T, 2),
        "lang_woT": _pack_kT(f("lang_wo").T, 2),
        "img_bt2": _bias2(f("img_bt")),
        "img_bp2": _bias2(f("img_bp")),
        "lang_bt2": _bias2(f("lang_bt")),
        "lang_bp2": _bias2(f("lang_bp")),
        "img_bo2": _bias2(f("img_wo") @ f("img_bv") + f("img_bo")),
        "lang_bo2": _bias2(f("lang_wo") @ f("lang_bv") + f("lang_bo")),
        "fus_b2": _bias2(f("fus_b")),
    }

    in_maps = []
    for c in range(8):
        b, half = c // 2, c % 2
        mm = np.concatenate(
            [images[b].reshape(256, N), flows[b].reshape(256, N), sp], axis=0)
        if half:
            mm = np.roll(mm, -NC, axis=1)
        in_maps.append({**base, "mm": np.ascontiguousarray(mm)})
    return in_maps


def kernel(**inputs):
    if "nc" not in _CACHE:
        _CACHE["nc"] = _build()
    nc = _CACHE["nc"]
    in_maps = _in_maps(inputs)
    res = run_bass_kernel_spmd(nc, in_maps, list(range(8)))
    out = np.empty((B, A, N), np.float32)
    for c in range(8):
        b, half = c // 2, c % 2
        out[b][:, half * NC:(half + 1) * NC] = res.results[c]["out"]
    return out.reshape(B, A, H, W)


# revision 15
# speedup vs baseline: 233.3435x; 197.0781x over previous
"""CMSA (cross-modal self-attention) model on 8 Trainium2 NeuronCores.

Model (B=4, C=256, H=W=64, N=4096, A=256):
  spatial = fixed 8-channel coordinate features            [B, 8, H, W]
  mm   = concat(images, flows, spatial)                    [B, 520, H, W]
  img_feat  = CMSA(mm,   img_w*)                           [B, 256, H, W]
  lang_feat = CMSA(flows, lang_w*)                         [B, 256, H, W]
  out = conv1x1(concat(img_feat, lang_feat, spatial), fus) [B, 256, H, W]
where CMSA(x) = wo @ softmax((wt@x)^T (wp@x)) applied to (wv@x), all 1x1 convs.

Sharding: 8 cores = 4 samples x 2 halves of the N=4096 pixel axis.  Each core
computes both CMSA branches and the fused output for its 2048 columns,
flash-attention style (full 4096x4096 attention rows never materialized in
HBM).  Attention is computed in the "transposed" orientation LT[m, n] so that
softmax needs no PE transposes: exp is taken without max-subtraction (logits
are bounded ~|15| for this model scale, safe in f32), the denominator is a
ones-matmul over partitions, and the value bias bv is folded into an effective
output bias bo_eff = wo@bv + bo using softmax row-sum normalization.

All matmuls run as float32r (full PE rate, ~1e-4 relative error).
"""

import numpy as np

import concourse.bass as bass
import concourse.tile as tile
import concourse.mybir as mybir
from concourse import bacc
from concourse.bass_utils import run_bass_kernel_spmd

F32 = mybir.dt.float32
F32R = mybir.dt.float32r
AF = mybir.ActivationFunctionType
ALU = mybir.AluOpType

B = 4
H = W = 64
N = H * W            # 4096
NC = N // 2          # columns per core
A = 256
C_MM = 520
NB = 512             # psum column block
NSB = NC // NB       # 4 blocks per core chunk
MT = N // 128        # 32 m-tiles
KI = 5               # k-tiles for C=520 (4x128 + 8)
KL = 2               # k-tiles for C=256

_CACHE = {}


def _emit(nc, tc, T):
    """Emit the per-core program. T maps dram tensor names -> APs."""
    ones_f32 = None

    # ---- pools ---------------------------------------------------------
    # left stack: whole-kernel consts | theta/phi/VT (img then lang, tag-shared)
    pL1 = tc.alloc_tile_pool(name="consts", bufs=1, side="left")
    pL2 = tc.alloc_tile_pool(name="abc", bufs=1, side="left")
    # right stack: R1 mm23+spc (to end of lang) | R2 mm01+sp+img qkv w (img
    # qkv only) | R3 working set (attention + tails)
    pR1 = tc.alloc_tile_pool(name="mm23", bufs=1, side="right")
    pR2 = tc.alloc_tile_pool(name="mm01", bufs=1, side="right")
    pps = tc.alloc_tile_pool(name="ps", bufs=1, space="PSUM")

    # ---- consts --------------------------------------------------------
    ones32 = pL1.tile([128, 1], F32, tag="ones32")
    nc.vector.memset(ones32, 1.0)
    ones_r = pL1.tile([128, 1], F32R, tag="ones")
    nc.scalar.copy(out=ones_r, in_=ones32)
    bias_t = {}
    for nm in ("img_bt2", "img_bp2", "lang_bt2", "lang_bp2",
               "img_bo2", "lang_bo2", "fus_b2"):
        t = pL1.tile([128, 2], F32, tag=nm, name=nm)
        nc.sync.dma_start(out=t, in_=T[nm])
        bias_t[nm] = t
    part_out = pL1.tile([128, 2, NC], F32, tag="part_out")

    # ---- big inputs ----------------------------------------------------
    # Load order matters for PE warmup: img qkv weights and the spatial rows
    # first (every qkv psum chain ends on them), then mm in column chunks
    # breadth-first so the first qkv tiles can start after ~1/4 of the load.
    imgw = {}
    CS = N // 4
    # mm lives as [128, CS] chunk tiles so DMA->compute deps are exact
    mm_cs = [[None] * 4 for _ in range(4)]   # [k][cs]
    for k in (2, 3):
        for cs in range(4):
            mm_cs[k][cs] = pR1.tile([128, CS], F32R, tag=f"mm{k}c{cs}",
                                    name=f"mm{k}c{cs}")
    spc = pR1.tile([8, NC], F32R, tag="spc")
    for k in (0, 1):
        for cs in range(4):
            mm_cs[k][cs] = pR2.tile([128, CS], F32R, tag=f"mm{k}c{cs}",
                                    name=f"mm{k}c{cs}")
    sp_sb = pR2.tile([8, N], F32R, tag="sp")
    for nm in ("img_wtT", "img_wpT", "img_wvT"):
        imgw[nm] = pR2.tile([128, KI, A], F32R, tag=nm, name=nm)

    def mm_cs_dma(cs):
        for k in range(4):
            nc.sync.dma_start(
                out=mm_cs[k][cs],
                in_=T["mm"][k * 128:(k + 1) * 128, cs * CS:(cs + 1) * CS].bitcast(F32R))

    nc.sync.dma_start(out=imgw["img_wtT"], in_=T["img_wtT"].bitcast(F32R))
    mm_cs_dma(0)
    nc.sync.dma_start(out=sp_sb, in_=T["mm"][512:520, :].bitcast(F32R))
    mm_cs_dma(1)
    nc.sync.dma_start(out=imgw["img_wpT"], in_=T["img_wpT"].bitcast(F32R))
    mm_cs_dma(2)
    nc.sync.dma_start(out=imgw["img_wvT"], in_=T["img_wvT"].bitcast(F32R))
    mm_cs_dma(3)

    def mm_ktile(k, cols):
        """[k-partitions, cols] slice of the mm operand for k-tile k.
        cols must lie within one CS-sized chunk for k < 4."""
        if k == 4:
            return sp_sb[:, cols]
        cs, lo, hi = cols.start // CS, cols.start % CS, None
        assert cols.stop - cols.start <= CS and cols.stop <= (cs + 1) * CS
        return mm_cs[k][cs][:, lo:lo + (cols.stop - cols.start)]

    def qkv_phase(branch, theta, phi, vt, wt, wp, wv, ks, bt2, bp2):
        """Computes theta [128,2,NC], phi [128,2,N], vt [128,MT,A] for one
        branch. ks = list of (ktile_idx, partitions)."""
        nk = len(ks)
        for a2 in range(2):
            asl = slice(a2 * 128, (a2 + 1) * 128)
            for ns in range(NSB):
                csl = slice(ns * NB, (ns + 1) * NB)
                q_ps = pps.tile([128, NB], F32, tag="blk", bufs=4, name="q_ps")
                for i, (k, kp) in enumerate(ks):
                    nc.tensor.matmul(q_ps, lhsT=wt[:kp, i, asl],
                                     rhs=mm_ktile(k, csl),
                                     start=(i == 0), stop=(i == nk - 1))
                nc.vector.tensor_scalar(out=theta[:, a2, csl], in0=q_ps,
                                        scalar1=bt2[:, a2:a2 + 1], scalar2=None,
                                        op0=ALU.add)
            for ns in range(N // NB):
                csl = slice(ns * NB, (ns + 1) * NB)
                q_ps = pps.tile([128, NB], F32, tag="blk", bufs=4, name="q_ps")
                for i, (k, kp) in enumerate(ks):
                    nc.tensor.matmul(q_ps, lhsT=wp[:kp, i, asl],
                                     rhs=mm_ktile(k, csl),
                                     start=(i == 0), stop=(i == nk - 1))
                nc.vector.tensor_scalar(out=phi[:, a2, csl], in0=q_ps,
                                        scalar1=bp2[:, a2:a2 + 1], scalar2=None,
                                        op0=ALU.add)
        for m in range(MT):
            msl = slice(m * 128, (m + 1) * 128)
            v_ps = pps.tile([128, A], F32, tag="blk", bufs=4, name="v_ps")
            for i, (k, kp) in enumerate(ks):
                nc.tensor.matmul(v_ps, lhsT=mm_ktile(k, msl)[:kp, :],
                                 rhs=wv[:kp, i, :],
                                 start=(i == 0), stop=(i == nk - 1))
            nc.vector.tensor_copy(out=vt[:, m, :], in_=v_ps)

    def attn_phase(branch, theta, phi, vt, pools):
        """Flash attention + wo conv + partial fusion for one branch."""
        woT = pools["woT_" + branch]
        fusT = pools["fusT"]
        bo2 = bias_t[f"{branch}_bo2"]
        pR3 = pools["pR3"]
        for nb in range(NSB):
            csl = slice(nb * NB, (nb + 1) * NB)
            att_ps = pps.tile([128, 2, NB], F32, tag="big2", bufs=2, name="att_ps")
            acc = pR3.tile([128, NB], F32R, tag="acc", bufs=2, name="acc")
            for m in range(MT):
                msl = slice(m * 128, (m + 1) * 128)
                lt_ps = pps.tile([128, NB], F32, tag="blk", bufs=4, name="lt_ps")
                for ka in range(2):
                    nc.tensor.matmul(lt_ps, lhsT=phi[:, ka, msl],
                                     rhs=theta[:, ka, csl],
                                     start=(ka == 0), stop=(ka == 1))
                p_sb = pR3.tile([128, NB], F32R, tag="p", bufs=5, name="p_sb")
                nc.scalar.activation(out=p_sb, in_=lt_ps, func=AF.Exp)
                if m == 0:
                    nc.vector.tensor_copy(out=acc, in_=p_sb)
                else:
                    nc.vector.tensor_add(out=acc, in0=acc, in1=p_sb)
                for a2 in range(2):
                    nc.tensor.matmul(att_ps[:, a2, :],
                                     lhsT=vt[:, m, a2 * 128:(a2 + 1) * 128],
                                     rhs=p_sb,
                                     start=(m == 0), stop=(m == MT - 1))
            # softmax denominator -> reciprocal -> broadcast
            rs_ps = pps.tile([128, NB], F32, tag="blk", bufs=4, name="rs_ps")
            nc.tensor.matmul(rs_ps[0:1, :], lhsT=ones_r, rhs=acc,
                             start=True, stop=True)
            rcp = pR3.tile([1, NB], F32, tag="rcp", bufs=2, name="rcp")
            nc.vector.reciprocal(out=rcp, in_=rs_ps[0:1, :])
            bc = pR3.tile([128, NB], F32, tag="bc", bufs=2, name="bc")
            nc.gpsimd.partition_broadcast(bc, rcp)
            # att (unnormalized) back to sbuf for the wo conv
            att_sb = pR3.tile([128, 2, NB], F32R, tag="att_sb", bufs=3, name="att_sb")
            for a2 in range(2):
                nc.scalar.copy(out=att_sb[:, a2, :], in_=att_ps[:, a2, :])
            wo_ps = pps.tile([128, 2, NB], F32, tag="big2", bufs=2, name="wo_ps")
            for o2 in range(2):
                for ka in range(2):
                    nc.tensor.matmul(wo_ps[:, o2, :],
                                     lhsT=woT[:, ka, o2 * 128:(o2 + 1) * 128],
                                     rhs=att_sb[:, ka, :],
                                     start=(ka == 0), stop=(ka == 1))
            feat = pR3.tile([128, 2, NB], F32R, tag="feat", bufs=2, name="feat")
            for o2 in range(2):
                t1 = pR3.tile([128, NB], F32, tag="t1", bufs=3, name="t1")
                nc.vector.tensor_tensor(out=t1, in0=wo_ps[:, o2, :], in1=bc, op=ALU.mult)
                nc.scalar.activation(out=feat[:, o2, :], in_=t1,
                                     func=AF.Identity, bias=bo2[:, o2:o2 + 1])
            # partial fusion
            f_ps = pps.tile([128, 2, NB], F32, tag="big2", bufs=2, name="f_ps")
            if branch == "img":
                for q2 in range(2):
                    qsl = slice(q2 * 128, (q2 + 1) * 128)
                    for k2 in range(2):
                        nc.tensor.matmul(f_ps[:, q2, :], lhsT=fusT[:, k2, qsl],
                                         rhs=feat[:, k2, :],
                                         start=(k2 == 0), stop=False)
                    nc.tensor.matmul(f_ps[:, q2, :], lhsT=fusT[:8, 4, qsl],
                                     rhs=spc[:, csl], start=False, stop=True)
                    nc.scalar.activation(out=part_out[:, q2, csl], in_=f_ps[:, q2, :],
                                         func=AF.Identity,
                                         bias=bias_t["fus_b2"][:, q2:q2 + 1])
            else:
                for q2 in range(2):
                    qsl = slice(q2 * 128, (q2 + 1) * 128)
                    for k2 in range(2):
                        nc.tensor.matmul(f_ps[:, q2, :], lhsT=fusT[:, 2 + k2, qsl],
                                         rhs=feat[:, k2, :],
                                         start=(k2 == 0), stop=(k2 == 1))
                    out_t = pR3.tile([128, NB], F32, tag="out_t", bufs=2, name="out_t")
                    nc.vector.tensor_tensor(out=out_t, in0=f_ps[:, q2, :],
                                            in1=part_out[:, q2, csl], op=ALU.add)
                    nc.sync.dma_start(
                        out=T["out"][q2 * 128:(q2 + 1) * 128, csl], in_=out_t)

    # ---- img qkv -------------------------------------------------------
    theta = pL2.tile([128, 2, NC], F32R, tag="theta", name="theta_i")
    phi = pL2.tile([128, 2, N], F32R, tag="phi", name="phi_i")
    vt = pL2.tile([128, MT, A], F32R, tag="vt", name="vt_i")
    ks_img = [(0, 128), (1, 128), (2, 128), (3, 128), (4, 8)]
    qkv_phase("img", theta, phi, vt, imgw["img_wtT"], imgw["img_wpT"],
              imgw["img_wvT"], ks_img, bias_t["img_bt2"], bias_t["img_bp2"])
    # stash this core's spatial columns before sp_sb dies with pR2
    nc.scalar.copy(out=spc, in_=sp_sb[:, 0:NC])
    pR2.release()

    # ---- working pool (attention + tails) ------------------------------
    pR3 = tc.alloc_tile_pool(name="work", bufs=1, side="right")
    pools = {"pR3": pR3}
    for nm, kt in (("woT_img", 2), ("woT_lang", 2), ("fusT", KI),
                   ("lang_wtT", KL), ("lang_wpT", KL), ("lang_wvT", KL)):
        dnm = {"woT_img": "img_woT", "woT_lang": "lang_woT"}.get(nm, nm)
        t = pR3.tile([128, kt, A], F32R, tag=nm, name=nm)
        nc.sync.dma_start(out=t, in_=T[dnm].bitcast(F32R))
        pools[nm] = t

    # ---- img attention + partial fusion --------------------------------
    attn_phase("img", theta, phi, vt, pools)

    # ---- lang qkv ------------------------------------------------------
    theta_l = pL2.tile([128, 2, NC], F32R, tag="theta", name="theta_l")
    phi_l = pL2.tile([128, 2, N], F32R, tag="phi", name="phi_l")
    vt_l = pL2.tile([128, MT, A], F32R, tag="vt", name="vt_l")
    ks_lang = [(2, 128), (3, 128)]
    qkv_phase("lang", theta_l, phi_l, vt_l, pools["lang_wtT"],
              pools["lang_wpT"], pools["lang_wvT"], ks_lang,
              bias_t["lang_bt2"], bias_t["lang_bp2"])

    # ---- lang attention + final output ---------------------------------
    attn_phase("lang", theta_l, phi_l, vt_l, pools)

    pR3.release()
    pR1.release()
    pL2.release()
    pL1.release()
    pps.release()


def _build(repeat=1):
    nc = bacc.Bacc("TRN2", target_bir_lowering=False, debug=False, num_devices=8)
    T = {}
    T["mm"] = nc.dram_tensor("mm", [C_MM, N], F32, kind="ExternalInput").ap()
    for nm in ("img_wtT", "img_wpT", "img_wvT", "fusT"):
        T[nm] = nc.dram_tensor(nm, [128, KI, A], F32, kind="ExternalInput").ap()
    for nm in ("lang_wtT", "lang_wpT", "lang_wvT"):
        T[nm] = nc.dram_tensor(nm, [128, KL, A], F32, kind="ExternalInput").ap()
    for nm in ("img_woT", "lang_woT"):
        T[nm] = nc.dram_tensor(nm, [128, 2, A], F32, kind="ExternalInput").ap()
    for nm in ("img_bt2", "img_bp2", "lang_bt2", "lang_bp2",
               "img_bo2", "lang_bo2", "fus_b2"):
        T[nm] = nc.dram_tensor(nm, [128, 2], F32, kind="ExternalInput").ap()
    T["out"] = nc.dram_tensor("out", [A, NC], F32, kind="ExternalOutput").ap()

    with tile.TileContext(nc) as tc:
        for _ in range(repeat):
            _emit(nc, tc, T)
    nc.compile()
    return nc


def _spatial():
    gy, gx = np.meshgrid(np.linspace(0, 1, H, dtype=np.float32),
                         np.linspace(0, 1, W, dtype=np.float32), indexing="ij")
    feats = [gx, gy, 1.0 - gx, 1.0 - gy] + [(gx + gy) * 0.5] * 4
    return np.stack(feats[:8], axis=0).reshape(8, N).astype(np.float32)


def _pack_kT(wT, kt):
    """[C, A] (pre-transposed weight) -> [128, kt, A] partition-tiled."""
    out = np.zeros((128, kt, wT.shape[1]), np.float32)
    for k in range(kt):
        rows = wT[k * 128:min((k + 1) * 128, wT.shape[0])]
        out[:rows.shape[0], k] = rows
    return out


def _bias2(b):
    return np.ascontiguousarray(b.reshape(2, 128).T)


def _in_maps(inputs):
    f = lambda k: np.asarray(inputs[k], np.float32)
    images, flows = f("images"), f("flows")
    sp = _spatial()

    base = {
        "img_wtT": _pack_kT(f("img_wt").T, KI),
        "img_wpT": _pack_kT(f("img_wp").T, KI),
        "img_wvT": _pack_kT(f("img_wv").T, KI),
        "fusT": _pack_kT(f("fus_w").T, KI),
        "lang_wtT": _pack_kT(f("lang_wt").T, KL),
        "lang_wpT": _pack_kT(f("lang_wp").T, KL),
        "lang_wvT": _pack_kT(f("lang_wv").T, KL),
        "img_woT": _pack_kT(f("img_wo").T, 2),
        "lang_woT": _pack_kT(f("lang_wo").T, 2),
        "img_bt2": _bias2(f("img_bt")),
        "img_bp2": _bias2(f("img_bp")),
        "lang_bt2": _bias2(f("lang_bt")),
        "lang_bp2": _bias2(f("lang_bp")),
        "img_bo2": _bias2(f("img_wo") @ f("img_bv") + f("img_bo")),
        "lang_bo2": _bias2(f("lang_wo") @ f("lang_bv") + f("lang_bo")),
        "fus_b2": _bias2(f("fus_b")),
    }

    in_maps = []
    for c in range(8):
        b, half = c // 2, c % 2
        mm = np.concatenate(
            [images[b].reshape(256, N), flows[b].reshape(256, N), sp], axis=0)
        if half:
            mm = np.roll(mm, -NC, axis=1)
        in_maps.append({**base, "mm": np.ascontiguousarray(mm)})
    return in_maps


def kernel(**inputs):
    if "nc" not in _CACHE:
        _CACHE["nc"] = _build()
    nc = _CACHE["nc"]
    in_maps = _in_maps(inputs)
    res = run_bass_kernel_spmd(nc, in_maps, list(range(8)))
    out = np.empty((B, A, N), np.float32)
    for c in range(8):
        b, half = c // 2, c % 2
        out[b][:, half * NC:(half + 1) * NC] = res.results[c]["out"]
    return out.reshape(B, A, H, W)
